# revision 14
# baseline (speedup 1.0000x reference)
"""AttentionBlock kernel for 8 Trainium2 NeuronCores.

Problem (hardcoded shapes): x [4, 256, 64, 64] f32.
  GroupNorm(32 groups) -> qkv 1x1 conv (768x256) -> 4-head attention over
  n=4096 tokens (hd=64) -> proj 1x1 conv -> residual add.

Sharding: 8 cores = (batch b in 0..3) x (query-half in 0..1).  Each core
computes GroupNorm + K/V for its whole batch image (duplicated across the
2 cores of a batch, cheap) and attention + proj + residual for its half of
the query positions (2048 of 4096).  Key order is permuted so the core's
query slice comes first; softmax is permutation-invariant over keys.

Per-core layout choices:
  - scores computed transposed: S_T[key_chunk(128 part), q(free)] =
    k_h^T q_h via TensorE (contraction = head_dim 64 on partitions).
  - exp on ScalarE directly PSUM->SBUF with the 1/sqrt(hd) scale folded in.
  - second matmul: out[d(+denom), q] = sum_kc (vT|ones)^T P, accumulating
    over key chunks in PSUM; the appended ones column yields the softmax
    denominator for free.
  - matmul inputs bitcast to float32r (1 cycle/row for N>=256).
"""

import sys

import numpy as np

sys.path.insert(0, "/opt/trn_rl_repo")

B, C, HW = 4, 256, 4096
NQ = HW // 2  # queries per core
NH, HD = 4, 64
G = 32  # groups
EPS = 1e-5

_CACHE = {}


def _build():
    import concourse.bass as bass
    import concourse.tile as tile
    from concourse import bacc, mybir

    f32 = mybir.dt.float32
    f32r = mybir.dt.float32r
    AF = mybir.ActivationFunctionType

    nc = bacc.Bacc(
        "TRN2",
        target_bir_lowering=False,
        debug=False,
        enable_asserts=False,
        num_devices=8,
    )

    x_d = nc.dram_tensor("x", [C, HW], f32, kind="ExternalInput").ap()
    qkv_wt_d = nc.dram_tensor("qkv_wt", [C, 3 * C], f32, kind="ExternalInput").ap()
    qkv_b_d = nc.dram_tensor("qkv_b", [3 * C], f32, kind="ExternalInput").ap()
    proj_wt_d = nc.dram_tensor("proj_wt", [C, C], f32, kind="ExternalInput").ap()
    proj_b_d = nc.dram_tensor("proj_b", [C], f32, kind="ExternalInput").ap()
    gn_w_d = nc.dram_tensor("gn_w", [C], f32, kind="ExternalInput").ap()
    gn_b_d = nc.dram_tensor("gn_b", [C], f32, kind="ExternalInput").ap()
    sel_d = nc.dram_tensor("sel", [128, 16], f32, kind="ExternalInput").ap()
    selT_d = nc.dram_tensor("selT", [16, 128], f32, kind="ExternalInput").ap()
    y_d = nc.dram_tensor("y", [C, NQ], f32, kind="ExternalOutput").ap()

    x_r = x_d.rearrange("(t p) n -> p t n", p=128)  # c = t*128 + p
    y_r = y_d.rearrange("(t p) n -> p t n", p=128)

    with tile.TileContext(nc) as tc:
        with (
            tc.tile_pool(name="const", bufs=1) as const,
            tc.tile_pool(name="big", bufs=1) as big,
            tc.tile_pool(name="work", bufs=2) as work,
            tc.tile_pool(name="pp", bufs=3) as pp,
            tc.tile_pool(name="psS", bufs=2, space="PSUM") as psS_pool,
            tc.tile_pool(name="psO", bufs=2, space="PSUM") as psO_pool,
        ):
            # ---- constants / weights ----
            wt_sb = const.tile([128, 2, 3 * C], f32r, tag="wt")
            nc.sync.dma_start(
                wt_sb, qkv_wt_d.rearrange("(t p) o -> p t o", p=128).bitcast(f32r)
            )
            wproj_sb = const.tile([64, NH, C], f32r, tag="wproj")
            nc.sync.dma_start(
                wproj_sb, proj_wt_d.rearrange("(h p) o -> p h o", p=64).bitcast(f32r)
            )
            qkvb_sb = const.tile([128, 6], f32, tag="qkvb")
            nc.sync.dma_start(qkvb_sb, qkv_b_d.rearrange("(s p) -> p s", p=128))
            vb_sb = const.tile([64, NH], f32, tag="vb")
            nc.sync.dma_start(vb_sb, qkv_b_d[2 * C :].rearrange("(h p) -> p h", p=64))
            projb_sb = const.tile([128, 2], f32, tag="projb")
            nc.sync.dma_start(projb_sb, proj_b_d.rearrange("(t p) -> p t", p=128))
            gnw_sb = const.tile([128, 2], f32, tag="gnw")
            nc.sync.dma_start(gnw_sb, gn_w_d.rearrange("(t p) -> p t", p=128))
            gnb_sb = const.tile([128, 2], f32, tag="gnb")
            nc.sync.dma_start(gnb_sb, gn_b_d.rearrange("(t p) -> p t", p=128))

            # group-selector matrices (channels<->groups), used for the tiny
            # cross-partition reductions in GroupNorm stats.
            ones1 = const.tile([1, 64], f32, tag="ones1")
            nc.vector.memset(ones1, 1.0)
            sel = const.tile([128, 16], f32, tag="sel")  # sel[p, g]=1 if p//8==g
            nc.sync.dma_start(sel, sel_d)
            selT = const.tile([16, 128], f32, tag="selT")
            nc.sync.dma_start(selT, selT_d)

            # ---- load x ----
            xs = big.tile([128, 2, HW], f32r, tag="xs")
            for t in range(2):
                for q4 in range(4):
                    nc.sync.dma_start(
                        xs[:, t, q4 * 1024 : (q4 + 1) * 1024],
                        x_r[:, t, q4 * 1024 : (q4 + 1) * 1024].bitcast(f32r),
                    )

            # ---- GroupNorm stats ----
            stats = const.tile([128, 2, 8, 6], f32, tag="stats")
            mv = const.tile([128, 2, 2], f32, tag="mv")
            for t in range(2):
                for jj in range(8):
                    nc.vector.bn_stats(
                        stats[:, t, jj, :],
                        xs[:, t, jj * 512 : (jj + 1) * 512].bitcast(f32),
                    )
                nc.vector.bn_aggr(mv[:, t, :], stats[:, t])
            # me2[:, t, 0] = mean_c ; me2[:, t, 1] = E[x^2]_c = var + mean^2
            me2 = const.tile([128, 2, 2], f32, tag="me2")
            for t in range(2):
                nc.vector.tensor_copy(me2[:, t, 0:1], mv[:, t, 0:1])
                nc.vector.tensor_mul(me2[:, t, 1:2], mv[:, t, 0:1], mv[:, t, 0:1])
                nc.vector.tensor_add(me2[:, t, 1:2], me2[:, t, 1:2], mv[:, t, 1:2])
            # group sums via selector matmul (fp32 for exactness)
            psg = psS_pool.tile([16, 2, 2], f32, tag="psS")
            for t in range(2):
                nc.tensor.matmul(
                    psg[:, t, :], lhsT=sel, rhs=me2[:, t, :], start=True, stop=True
                )
            gsb = const.tile([16, 2, 2], f32, tag="gsb")
            gmr = const.tile([16, 2, 2], f32, tag="gmr")  # (mean_g, rstd_g)
            eps_sb = const.tile([16, 1], f32, tag="eps")
            nc.vector.memset(eps_sb, EPS)
            for t in range(2):
                nc.vector.tensor_scalar_mul(gsb[:, t, :], psg[:, t, :], 1.0 / 8.0)
                nc.vector.tensor_copy(gmr[:, t, 0:1], gsb[:, t, 0:1])
                # var_g = E2_g - mean_g^2
                nc.vector.tensor_mul(gmr[:, t, 1:2], gsb[:, t, 0:1], gsb[:, t, 0:1])
                nc.vector.tensor_sub(gmr[:, t, 1:2], gsb[:, t, 1:2], gmr[:, t, 1:2])
                nc.scalar.activation(
                    gmr[:, t, 1:2], gmr[:, t, 1:2], AF.Sqrt, bias=eps_sb, scale=1.0
                )
                nc.vector.reciprocal(gmr[:, t, 1:2], gmr[:, t, 1:2])
            # broadcast group stats back to channels
            psb = psS_pool.tile([128, 2, 2], f32, tag="psS")
            for t in range(2):
                nc.tensor.matmul(
                    psb[:, t, :], lhsT=selT, rhs=gmr[:, t, :], start=True, stop=True
                )
            # per-channel affine: xn = x * a + bcoef
            ab = const.tile([128, 2, 2], f32, tag="ab")
            for t in range(2):
                nc.vector.tensor_mul(ab[:, t, 0:1], psb[:, t, 1:2], gnw_sb[:, t : t + 1])
                nc.vector.tensor_mul(ab[:, t, 1:2], psb[:, t, 0:1], ab[:, t, 0:1])
                nc.vector.tensor_sub(ab[:, t, 1:2], gnb_sb[:, t : t + 1], ab[:, t, 1:2])
            for t in range(2):
                nc.vector.tensor_scalar(
                    xs[:, t, :],
                    xs[:, t, :].bitcast(f32),
                    scalar1=ab[:, t, 0:1],
                    scalar2=ab[:, t, 1:2],
                    op0=mybir.AluOpType.mult,
                    op1=mybir.AluOpType.add,
                )

            # ---- QKV projections ----
            # k2/q2: [128, pair, n] with head (2*pair + p//64) at partition
            # (p%64); produced directly by 128-wide output matmuls.
            k2 = big.tile([128, 2, HW], f32r, tag="k2")
            q2 = big.tile([128, 2, NQ], f32r, tag="q2")
            # vT1: [key_chunk_part, kc, h, 65]; col 64 = ones (denominator).
            vT1 = big.tile([128, 32, NH, 65], f32r, tag="vT1")
            onesc = const.tile([128, 1], f32, tag="onesc")
            nc.vector.memset(onesc, 1.0)
            nc.vector.tensor_copy(
                vT1[:, :, :, 64:65], onesc.to_broadcast((128, 32, NH, 1))
            )

            for pair in range(2):
                for n8 in range(8):
                    ps = psS_pool.tile([128, 512], f32, tag="psS")
                    for t in range(2):
                        nc.tensor.matmul(
                            ps,
                            lhsT=wt_sb[:, t, C + pair * 128 : C + (pair + 1) * 128].bitcast(f32r),
                            rhs=xs[:, t, n8 * 512 : (n8 + 1) * 512].bitcast(f32r),
                            start=(t == 0),
                            stop=(t == 1),
                        )
                    nc.vector.tensor_scalar_add(
                        k2[:, pair, n8 * 512 : (n8 + 1) * 512], ps,
                        qkvb_sb[:, 2 + pair : 3 + pair],
                    )
            for pair in range(2):
                for j4 in range(4):
                    ps = psS_pool.tile([128, 512], f32, tag="psS")
                    for t in range(2):
                        nc.tensor.matmul(
                            ps,
                            lhsT=wt_sb[:, t, pair * 128 : (pair + 1) * 128].bitcast(f32r),
                            rhs=xs[:, t, j4 * 512 : (j4 + 1) * 512].bitcast(f32r),
                            start=(t == 0),
                            stop=(t == 1),
                        )
                    nc.vector.tensor_scalar_add(
                        q2[:, pair, j4 * 512 : (j4 + 1) * 512], ps,
                        qkvb_sb[:, pair : pair + 1],
                    )
            for kc in range(32):
                ps = psS_pool.tile([128, 256], f32, tag="psS")
                for t in range(2):
                    nc.tensor.matmul(
                        ps,
                        lhsT=xs[:, t, kc * 128 : (kc + 1) * 128].bitcast(f32r),
                        rhs=wt_sb[:, t, 2 * C : 3 * C].bitcast(f32r),
                        start=(t == 0),
                        stop=(t == 1),
                    )
                nc.vector.tensor_copy(
                    vT1[:, kc, :, 0:64], ps.rearrange("p (h d) -> p h d", h=NH)
                )

            # ---- attention + proj per query tile ----
            groups = [3] * 10 + [2]  # 32 key chunks in ACT-sized groups
            for j in range(4):
                att_j = work.tile([64, NH, 512], f32, tag="att")
                for h in range(4):
                    pair, hp = h // 2, h % 2
                    lo, hi = hp * 64, hp * 64 + 64
                    psO = psO_pool.tile([65, 512], f32, tag="psO")
                    kc0 = 0
                    for glen in groups:
                        psS = psS_pool.tile([128, 3, 512], f32, tag="psS")
                        for i in range(glen):
                            kc = kc0 + i
                            nc.tensor.matmul(
                                psS[:, i, :],
                                lhsT=k2[lo:hi, pair, kc * 128 : (kc + 1) * 128].bitcast(f32r),
                                rhs=q2[lo:hi, pair, j * 512 : (j + 1) * 512].bitcast(f32r),
                                start=True,
                                stop=True,
                            )
                        P = pp.tile([128, 3, 512], f32r, tag="P")
                        nc.scalar.activation(
                            P[:, :glen, :], psS[:, :glen, :], AF.Exp,
                            scale=float(HD) ** -0.5,
                        )
                        for i in range(glen):
                            kc = kc0 + i
                            nc.tensor.matmul(
                                psO,
                                lhsT=vT1[:, kc, h, :].bitcast(f32r),
                                rhs=P[:, i, :].bitcast(f32r),
                                start=(kc == 0),
                                stop=(kc == 31),
                            )
                        kc0 += glen
                    rec = work.tile([1, 512], f32, tag="rec")
                    nc.vector.reciprocal(rec, psO[64:65, :])
                    # broadcast rec across partitions: ones[1,64]^T @ rec[1,512]
                    psB = psO_pool.tile([64, 512], f32, tag="psO")
                    nc.tensor.matmul(psB, lhsT=ones1, rhs=rec, start=True, stop=True)
                    rec_b = work.tile([64, 512], f32, tag="recb")
                    nc.vector.tensor_copy(rec_b, psB)
                    nc.vector.tensor_mul(
                        att_j[:, h, :].bitcast(f32r), psO[0:64, :], rec_b
                    )
                    nc.vector.tensor_scalar_add(
                        att_j[:, h, :].bitcast(f32r), att_j[:, h, :], vb_sb[:, h : h + 1]
                    )
                # proj + bias + residual for this query tile
                y_sb = work.tile([128, 2, 512], f32, tag="y")
                xres = work.tile([128, 2, 512], f32, tag="xres")
                nc.sync.dma_start(xres, x_r[:, :, j * 512 : (j + 1) * 512])
                for ot in range(2):
                    psY = psS_pool.tile([128, 512], f32, tag="psS")
                    for h in range(4):
                        nc.tensor.matmul(
                            psY,
                            lhsT=wproj_sb[:, h, ot * 128 : (ot + 1) * 128].bitcast(f32r),
                            rhs=att_j[:, h, :].bitcast(f32r),
                            start=(h == 0),
                            stop=(h == 3),
                        )
                    nc.vector.tensor_scalar_add(
                        y_sb[:, ot, :], psY, projb_sb[:, ot : ot + 1]
                    )
                    nc.vector.tensor_add(y_sb[:, ot, :], y_sb[:, ot, :], xres[:, ot, :])
                nc.sync.dma_start(y_r[:, :, j * 512 : (j + 1) * 512], y_sb)

    nc.compile()
    return nc


def _get_program():
    if "nc" not in _CACHE:
        _CACHE["nc"] = _build()
    return _CACHE["nc"]


def kernel(x, gn_w, gn_b, qkv_w, qkv_b, proj_w, proj_b):
    from concourse.bass_utils import run_bass_kernel_spmd

    x = np.asarray(x, np.float32)
    gn_w = np.asarray(gn_w, np.float32)
    gn_b = np.asarray(gn_b, np.float32)
    qkv_w = np.asarray(qkv_w, np.float32)
    qkv_b = np.asarray(qkv_b, np.float32)
    proj_w = np.asarray(proj_w, np.float32)
    proj_b = np.asarray(proj_b, np.float32)

    nc = _get_program()
    qkv_wt = np.ascontiguousarray(qkv_w.T)
    proj_wt = np.ascontiguousarray(proj_w.T)
    sel = np.zeros((128, 16), np.float32)
    sel[np.arange(128), np.arange(128) // 8] = 1.0
    selT = np.ascontiguousarray(sel.T)

    in_maps = []
    for core in range(8):
        b, half = core // 2, core % 2
        xb = x[b].reshape(C, HW)
        if half == 1:
            xb = np.concatenate([xb[:, NQ:], xb[:, :NQ]], axis=1)
        in_maps.append(
            {
                "x": np.ascontiguousarray(xb),
                "qkv_wt": qkv_wt,
                "qkv_b": qkv_b,
                "proj_wt": proj_wt,
                "proj_b": proj_b,
                "gn_w": gn_w,
                "gn_b": gn_b,
                "sel": sel,
                "selT": selT,
            }
        )

    res = run_bass_kernel_spmd(nc, in_maps, core_ids=list(range(8)))
    out = np.empty((B, C, HW), np.float32)
    for core in range(8):
        b, half = core // 2, core % 2
        out[b][:, half * NQ : (half + 1) * NQ] = res.results[core]["y"]
    return out.reshape(B, C, 64, 64)


# revision 15
# speedup vs baseline: 1.2141x; 1.2141x over previous
"""AttentionBlock kernel for 8 Trainium2 NeuronCores.

Problem (hardcoded shapes): x [4, 256, 64, 64] f32.
  GroupNorm(32 groups) -> qkv 1x1 conv (768x256) -> 4-head attention over
  n=4096 tokens (hd=64) -> proj 1x1 conv -> residual add.

Sharding: 8 cores = (batch b in 0..3) x (query-half in 0..1).  Each core
computes GroupNorm + K/V for its whole batch image (duplicated across the
2 cores of a batch, cheap) and attention + proj + residual for its half of
the query positions (2048 of 4096).  Key order is permuted so the core's
query slice comes first; softmax is permutation-invariant over keys.

Per-core layout choices:
  - scores computed transposed: S_T[key_chunk(128 part), q(free)] =
    k_h^T q_h via TensorE (contraction = head_dim 64 on partitions).
  - exp on ScalarE directly PSUM->SBUF with the 1/sqrt(hd) scale folded in.
  - second matmul: out[d(+denom), q] = sum_kc (vT|ones)^T P, accumulating
    over key chunks in PSUM; the appended ones column yields the softmax
    denominator for free.
  - attention matmul operands in bf16 (1 cycle/row on PE); GroupNorm
    stats, softmax denominator/division, and the residual add stay fp32.
"""

import sys

import numpy as np

sys.path.insert(0, "/opt/trn_rl_repo")

B, C, HW = 4, 256, 4096
NQ = HW // 2  # queries per core
NH, HD = 4, 64
G = 32  # groups
EPS = 1e-5

_CACHE = {}


def _build():
    import concourse.bass as bass
    import concourse.tile as tile
    from concourse import bacc, mybir

    f32 = mybir.dt.float32
    bf16 = mybir.dt.bfloat16
    AF = mybir.ActivationFunctionType

    nc = bacc.Bacc(
        "TRN2",
        target_bir_lowering=False,
        debug=False,
        enable_asserts=False,
        num_devices=8,
    )

    x_d = nc.dram_tensor("x", [C, HW], f32, kind="ExternalInput").ap()
    qkv_wt_d = nc.dram_tensor("qkv_wt", [C, 3 * C], bf16, kind="ExternalInput").ap()
    qkv_b_d = nc.dram_tensor("qkv_b", [3 * C], f32, kind="ExternalInput").ap()
    proj_wt_d = nc.dram_tensor("proj_wt", [C, C], bf16, kind="ExternalInput").ap()
    proj_b_d = nc.dram_tensor("proj_b", [C], f32, kind="ExternalInput").ap()
    gn_w_d = nc.dram_tensor("gn_w", [C], f32, kind="ExternalInput").ap()
    gn_b_d = nc.dram_tensor("gn_b", [C], f32, kind="ExternalInput").ap()
    sel_d = nc.dram_tensor("sel", [128, 16], f32, kind="ExternalInput").ap()
    selT_d = nc.dram_tensor("selT", [16, 128], f32, kind="ExternalInput").ap()
    y_d = nc.dram_tensor("y", [C, NQ], f32, kind="ExternalOutput").ap()

    x_r = x_d.rearrange("(t p) n -> p t n", p=128)  # c = t*128 + p
    y_r = y_d.rearrange("(t p) n -> p t n", p=128)

    with tile.TileContext(nc) as tc:
        with (
            tc.tile_pool(name="const", bufs=1) as const,
            tc.tile_pool(name="big", bufs=1) as big,
            tc.tile_pool(name="work", bufs=2) as work,
            tc.tile_pool(name="pp", bufs=3) as pp,
            tc.tile_pool(name="psS", bufs=2, space="PSUM") as psS_pool,
            tc.tile_pool(name="psO", bufs=2, space="PSUM") as psO_pool,
        ):
            # ---- constants / weights ----
            wt_sb = const.tile([128, 2, 3 * C], bf16, tag="wt")
            nc.sync.dma_start(wt_sb, qkv_wt_d.rearrange("(t p) o -> p t o", p=128))
            wproj_sb = const.tile([64, NH, C], bf16, tag="wproj")
            nc.sync.dma_start(wproj_sb, proj_wt_d.rearrange("(h p) o -> p h o", p=64))
            qkvb_sb = const.tile([128, 6], f32, tag="qkvb")
            nc.sync.dma_start(qkvb_sb, qkv_b_d.rearrange("(s p) -> p s", p=128))
            vb_sb = const.tile([64, NH], f32, tag="vb")
            nc.sync.dma_start(vb_sb, qkv_b_d[2 * C :].rearrange("(h p) -> p h", p=64))
            projb_sb = const.tile([128, 2], f32, tag="projb")
            nc.sync.dma_start(projb_sb, proj_b_d.rearrange("(t p) -> p t", p=128))
            gnw_sb = const.tile([128, 2], f32, tag="gnw")
            nc.sync.dma_start(gnw_sb, gn_w_d.rearrange("(t p) -> p t", p=128))
            gnb_sb = const.tile([128, 2], f32, tag="gnb")
            nc.sync.dma_start(gnb_sb, gn_b_d.rearrange("(t p) -> p t", p=128))

            # group-selector matrices (channels<->groups), used for the tiny
            # cross-partition reductions in GroupNorm stats.
            ones1 = const.tile([1, 64], f32, tag="ones1")
            nc.vector.memset(ones1, 1.0)
            sel = const.tile([128, 16], f32, tag="sel")  # sel[p, g]=1 if p//8==g
            nc.sync.dma_start(sel, sel_d)
            selT = const.tile([16, 128], f32, tag="selT")
            nc.sync.dma_start(selT, selT_d)

            # ---- load x ----
            xs = big.tile([128, 2, HW], f32, tag="xs")
            for t in range(2):
                for q4 in range(4):
                    nc.sync.dma_start(
                        xs[:, t, q4 * 1024 : (q4 + 1) * 1024],
                        x_r[:, t, q4 * 1024 : (q4 + 1) * 1024],
                    )

            # ---- GroupNorm stats ----
            stats = const.tile([128, 2, 8, 6], f32, tag="stats")
            mv = const.tile([128, 2, 2], f32, tag="mv")
            for t in range(2):
                for jj in range(8):
                    nc.vector.bn_stats(
                        stats[:, t, jj, :], xs[:, t, jj * 512 : (jj + 1) * 512]
                    )
                nc.vector.bn_aggr(mv[:, t, :], stats[:, t])
            # me2[:, t, 0] = mean_c ; me2[:, t, 1] = E[x^2]_c = var + mean^2
            me2 = const.tile([128, 2, 2], f32, tag="me2")
            for t in range(2):
                nc.vector.tensor_copy(me2[:, t, 0:1], mv[:, t, 0:1])
                nc.vector.tensor_mul(me2[:, t, 1:2], mv[:, t, 0:1], mv[:, t, 0:1])
                nc.vector.tensor_add(me2[:, t, 1:2], me2[:, t, 1:2], mv[:, t, 1:2])
            # group sums via selector matmul (fp32 for exactness)
            psg = psS_pool.tile([16, 2, 2], f32, tag="psS")
            for t in range(2):
                nc.tensor.matmul(
                    psg[:, t, :], lhsT=sel, rhs=me2[:, t, :], start=True, stop=True
                )
            gsb = const.tile([16, 2, 2], f32, tag="gsb")
            gmr = const.tile([16, 2, 2], f32, tag="gmr")  # (mean_g, rstd_g)
            eps_sb = const.tile([16, 1], f32, tag="eps")
            nc.vector.memset(eps_sb, EPS)
            for t in range(2):
                nc.vector.tensor_scalar_mul(gsb[:, t, :], psg[:, t, :], 1.0 / 8.0)
                nc.vector.tensor_copy(gmr[:, t, 0:1], gsb[:, t, 0:1])
                # var_g = E2_g - mean_g^2
                nc.vector.tensor_mul(gmr[:, t, 1:2], gsb[:, t, 0:1], gsb[:, t, 0:1])
                nc.vector.tensor_sub(gmr[:, t, 1:2], gsb[:, t, 1:2], gmr[:, t, 1:2])
                nc.scalar.activation(
                    gmr[:, t, 1:2], gmr[:, t, 1:2], AF.Sqrt, bias=eps_sb, scale=1.0
                )
                nc.vector.reciprocal(gmr[:, t, 1:2], gmr[:, t, 1:2])
            # broadcast group stats back to channels
            psb = psS_pool.tile([128, 2, 2], f32, tag="psS")
            for t in range(2):
                nc.tensor.matmul(
                    psb[:, t, :], lhsT=selT, rhs=gmr[:, t, :], start=True, stop=True
                )
            # per-channel affine: xn = x * a + bcoef  (xn in bf16 for PE)
            ab = const.tile([128, 2, 2], f32, tag="ab")
            for t in range(2):
                nc.vector.tensor_mul(ab[:, t, 0:1], psb[:, t, 1:2], gnw_sb[:, t : t + 1])
                nc.vector.tensor_mul(ab[:, t, 1:2], psb[:, t, 0:1], ab[:, t, 0:1])
                nc.vector.tensor_sub(ab[:, t, 1:2], gnb_sb[:, t : t + 1], ab[:, t, 1:2])
            xn = big.tile([128, 2, HW], bf16, tag="xn")
            for t in range(2):
                for q4 in range(4):
                    nc.vector.tensor_scalar(
                        xn[:, t, q4 * 1024 : (q4 + 1) * 1024],
                        xs[:, t, q4 * 1024 : (q4 + 1) * 1024],
                        scalar1=ab[:, t, 0:1],
                        scalar2=ab[:, t, 1:2],
                        op0=mybir.AluOpType.mult,
                        op1=mybir.AluOpType.add,
                    )

            # ---- QKV projections ----
            # k2/q2: [128, pair, n] with head (2*pair + p//64) at partition
            # (p%64); produced directly by 128-wide output matmuls.
            k2 = big.tile([128, 2, HW], bf16, tag="k2")
            q2 = big.tile([128, 2, NQ], bf16, tag="q2")
            # vT1: [key_chunk_part, kc, h, 65]; col 64 = ones (denominator).
            vT1 = big.tile([128, 32, NH, 65], bf16, tag="vT1")
            onesc = const.tile([128, 1], f32, tag="onesc")
            nc.vector.memset(onesc, 1.0)
            nc.vector.tensor_copy(
                vT1[:, :, :, 64:65], onesc.to_broadcast((128, 32, NH, 1))
            )

            for pair in range(2):
                for n8 in range(8):
                    ps = psS_pool.tile([128, 512], f32, tag="psS")
                    for t in range(2):
                        nc.tensor.matmul(
                            ps,
                            lhsT=wt_sb[:, t, C + pair * 128 : C + (pair + 1) * 128],
                            rhs=xn[:, t, n8 * 512 : (n8 + 1) * 512],
                            start=(t == 0),
                            stop=(t == 1),
                        )
                    nc.vector.tensor_scalar_add(
                        k2[:, pair, n8 * 512 : (n8 + 1) * 512], ps,
                        qkvb_sb[:, 2 + pair : 3 + pair],
                    )
            for pair in range(2):
                for j4 in range(4):
                    ps = psS_pool.tile([128, 512], f32, tag="psS")
                    for t in range(2):
                        nc.tensor.matmul(
                            ps,
                            lhsT=wt_sb[:, t, pair * 128 : (pair + 1) * 128],
                            rhs=xn[:, t, j4 * 512 : (j4 + 1) * 512],
                            start=(t == 0),
                            stop=(t == 1),
                        )
                    nc.vector.tensor_scalar_add(
                        q2[:, pair, j4 * 512 : (j4 + 1) * 512], ps,
                        qkvb_sb[:, pair : pair + 1],
                    )
            for kc in range(32):
                ps = psS_pool.tile([128, 256], f32, tag="psS")
                for t in range(2):
                    nc.tensor.matmul(
                        ps,
                        lhsT=xn[:, t, kc * 128 : (kc + 1) * 128],
                        rhs=wt_sb[:, t, 2 * C : 3 * C],
                        start=(t == 0),
                        stop=(t == 1),
                    )
                nc.vector.tensor_copy(
                    vT1[:, kc, :, 0:64], ps.rearrange("p (h d) -> p h d", h=NH)
                )

            # ---- attention + proj per query tile ----
            groups = [3] * 10 + [2]  # 32 key chunks in ACT-sized groups
            for j in range(4):
                att_j = work.tile([64, NH, 512], bf16, tag="att")
                for h in range(4):
                    pair, hp = h // 2, h % 2
                    lo, hi = hp * 64, hp * 64 + 64
                    psO = psO_pool.tile([65, 512], f32, tag="psO")
                    kc0 = 0
                    for glen in groups:
                        psS = psS_pool.tile([128, 3, 512], f32, tag="psS")
                        for i in range(glen):
                            kc = kc0 + i
                            nc.tensor.matmul(
                                psS[:, i, :],
                                lhsT=k2[lo:hi, pair, kc * 128 : (kc + 1) * 128],
                                rhs=q2[lo:hi, pair, j * 512 : (j + 1) * 512],
                                start=True,
                                stop=True,
                            )
                        P = pp.tile([128, 3, 512], bf16, tag="P")
                        nc.scalar.activation(
                            P[:, :glen, :], psS[:, :glen, :], AF.Exp,
                            scale=float(HD) ** -0.5,
                        )
                        for i in range(glen):
                            kc = kc0 + i
                            nc.tensor.matmul(
                                psO,
                                lhsT=vT1[:, kc, h, :],
                                rhs=P[:, i, :],
                                start=(kc == 0),
                                stop=(kc == 31),
                            )
                        kc0 += glen
                    rec = work.tile([1, 512], f32, tag="rec")
                    nc.vector.reciprocal(rec, psO[64:65, :])
                    # broadcast rec across partitions: ones[1,64]^T @ rec[1,512]
                    psB = psO_pool.tile([64, 512], f32, tag="psO")
                    nc.tensor.matmul(psB, lhsT=ones1, rhs=rec, start=True, stop=True)
                    rec_b = work.tile([64, 512], f32, tag="recb")
                    nc.vector.tensor_copy(rec_b, psB)
                    nc.vector.tensor_mul(att_j[:, h, :], psO[0:64, :], rec_b)
                    nc.vector.tensor_scalar_add(
                        att_j[:, h, :], att_j[:, h, :], vb_sb[:, h : h + 1]
                    )
                # proj + bias + residual for this query tile
                y_sb = work.tile([128, 2, 512], f32, tag="y")
                xres = work.tile([128, 2, 512], f32, tag="xres")
                nc.sync.dma_start(xres, x_r[:, :, j * 512 : (j + 1) * 512])
                for ot in range(2):
                    psY = psS_pool.tile([128, 512], f32, tag="psS")
                    for h in range(4):
                        nc.tensor.matmul(
                            psY,
                            lhsT=wproj_sb[:, h, ot * 128 : (ot + 1) * 128],
                            rhs=att_j[:, h, :],
                            start=(h == 0),
                            stop=(h == 3),
                        )
                    nc.vector.tensor_scalar_add(
                        y_sb[:, ot, :], psY, projb_sb[:, ot : ot + 1]
                    )
                    nc.vector.tensor_add(y_sb[:, ot, :], y_sb[:, ot, :], xres[:, ot, :])
                nc.sync.dma_start(y_r[:, :, j * 512 : (j + 1) * 512], y_sb)

    nc.compile()
    return nc


def _get_program():
    if "nc" not in _CACHE:
        _CACHE["nc"] = _build()
    return _CACHE["nc"]


def kernel(x, gn_w, gn_b, qkv_w, qkv_b, proj_w, proj_b):
    import ml_dtypes

    from concourse.bass_utils import run_bass_kernel_spmd

    x = np.asarray(x, np.float32)
    gn_w = np.asarray(gn_w, np.float32)
    gn_b = np.asarray(gn_b, np.float32)
    qkv_w = np.asarray(qkv_w, np.float32)
    qkv_b = np.asarray(qkv_b, np.float32)
    proj_w = np.asarray(proj_w, np.float32)
    proj_b = np.asarray(proj_b, np.float32)

    nc = _get_program()
    qkv_wt = np.ascontiguousarray(qkv_w.T).astype(ml_dtypes.bfloat16)
    proj_wt = np.ascontiguousarray(proj_w.T).astype(ml_dtypes.bfloat16)
    sel = np.zeros((128, 16), np.float32)
    sel[np.arange(128), np.arange(128) // 8] = 1.0
    selT = np.ascontiguousarray(sel.T)

    in_maps = []
    for core in range(8):
        b, half = core // 2, core % 2
        xb = x[b].reshape(C, HW)
        if half == 1:
            xb = np.concatenate([xb[:, NQ:], xb[:, :NQ]], axis=1)
        in_maps.append(
            {
                "x": np.ascontiguousarray(xb),
                "qkv_wt": qkv_wt,
                "qkv_b": qkv_b,
                "proj_wt": proj_wt,
                "proj_b": proj_b,
                "gn_w": gn_w,
                "gn_b": gn_b,
                "sel": sel,
                "selT": selT,
            }
        )

    res = run_bass_kernel_spmd(nc, in_maps, core_ids=list(range(8)))
    out = np.empty((B, C, HW), np.float32)
    for core in range(8):
        b, half = core // 2, core % 2
        out[b][:, half * NQ : (half + 1) * NQ] = res.results[core]["y"]
    return out.reshape(B, C, 64, 64)
